# revision 30
# baseline (speedup 1.0000x reference)
"""Trainium2 Bass kernel for nn_BodyModelTorch (SMPL-style body model).

Strategy (pure data parallel, batch sharded across 8 cores):
  - Per core: B_shard=32 bodies on 96 partitions laid out as p = 3*b + i
    (i = coordinate row), so per-partition scalars (scale, trans) and the
    final output layout fall out naturally.
  - Forward kinematics on device: ACT sin/cos (tanh for bone factors -- one
    shared ACT table set), DVE rotation-matrix entries, one-hot matmul for
    the bone-length factor gather (+ offset scaling), 5-level binary-tree
    compose with partition-replicated A matrices.
  - LBS skinning as a single 256-deep (248 + zero pad) contraction:
        v[b,i,v'] = sum_{k,c} G''[(k,c),(b,i)] * WT'[(k,c),v']
    where WT'[(k,c),v'] = weights[v',k] * rest_h[v',c] is batch-independent
    and precomputed on the host; scale/trans/inverse-bind are folded into
    G'' so there is no epilogue.
  - WT' ships as fp16 (rel err ~1e-4; set WT_FP32=True for full fp32).
  - All big DMAs use contiguous DRAM blocks shaped [128, cols] so the HWDGE
    sprays descriptors over all 16 SDMA engines (strided 2D DRAM transfers
    pin to 4 engines = ~54 GB/s).
  - Loads ride the SP HWDGE ring (nc.sync), stores the ACT ring (nc.scalar).
  - Device output is [96, V] / [96, K] piece-blocked; the host reorders to
    [B, V, 3] / [B, K, 3] while unsharding.
"""

import numpy as np

import concourse.bass as bass
import concourse.tile as tile
from concourse import mybir
from concourse.bass_utils import run_bass_kernel_spmd

F32 = mybir.dt.float32
AF = mybir.ActivationFunctionType

WT_FP32 = False
WT_DT = F32 if WT_FP32 else mybir.dt.float16
WT_NP = np.float32 if WT_FP32 else np.float16

B, K, V, NB = 256, 62, 14522, 28
NCORES = 8
BS = B // NCORES            # 32 bodies per core
P96 = 3 * BS                # 96 partitions: p = 3*b + i
C248 = 4 * K                # contraction dim (k, c)
HALF = C248 // 2            # 124 = 31 joints * 4
PADC = 128                  # contraction rows per half (zero-padded)
CHUNK = 512
NCHUNK = (V + CHUNK - 1) // CHUNK            # 29
# (start, count, parent_start) per tree level; parents[j] = (j-1)//2.
# level 5 is padded to 32 joints (phantom joint 62 writes scratch).
LEVELS = [(1, 2, 0), (3, 4, 1), (7, 8, 3), (15, 16, 7), (31, 32, 15)]
KP = K + 1                  # joint-padded buffers
WT_PIECES = [0, 2048, 4096, 6144, 8192, 10240, 12288, V]
# output pieces in chunk indices; each piece is its own SBUF tile and its
# own contiguous [96, cols] DRAM block
OUT_PIECES = [(c, min(c + 2, NCHUNK)) for c in range(0, NCHUNK, 2)]


def _split_sync_waits(nc, maxw=1):
    """The walrus build here accepts only one sync-wait command per
    instruction; move surplus waits onto preceding same-engine NOPs."""
    for f in nc.m.functions:
        for bb in f.blocks:
            insts = list(bb.instructions)
            out = []
            changed = False
            for ins in insts:
                si = ins.sync_info
                if si is not None and si.on_wait and len(si.on_wait) > maxw:
                    waits = list(si.on_wait)
                    extra, keep = waits[:-maxw], waits[-maxw:]
                    for j, w in enumerate(extra):
                        out.append(mybir.InstNoOp(
                            name=f"{ins.name}-wsplit{j}",
                            sync_info=mybir.SyncInfo(on_wait=[w], on_update=[]),
                            bass_nofuse=True,
                            engine=ins.engine,
                        ))
                    si.on_wait = keep
                    changed = True
                out.append(ins)
            if changed:
                bb.instructions[:] = out


PA_COLS = 3 * K + 3 * K + 1 + 1 + 4 + P96    # th|jt|scale|trans|idrow|I96
PB_COLS = P96 + 3 * K                         # fpack | mmap


def _build():
    nc = bass.Bass("TRN2")
    d_pa = nc.dram_tensor("pack_a", [P96, PA_COLS], F32, kind="ExternalInput")
    d_pb = nc.dram_tensor("pack_b", [NB + 2, PB_COLS], F32, kind="ExternalInput")
    d_w0 = nc.dram_tensor("wt0", [PADC * V], WT_DT, kind="ExternalInput")
    d_w1 = nc.dram_tensor("wt1", [PADC * V], WT_DT, kind="ExternalInput")
    d_vo = nc.dram_tensor("v_out", [P96 * V], F32, kind="ExternalOutput")
    d_jo = nc.dram_tensor("j_out", [P96, K], F32, kind="ExternalOutput")

    with tile.TileContext(nc) as tc:
        with (
            tc.tile_pool(name="big", bufs=1) as big,
            tc.tile_pool(name="fk", bufs=1) as fk,
            tc.tile_pool(name="tmp", bufs=1) as tmp,
            tc.tile_pool(name="ps1", bufs=1, space="PSUM") as ps1,
            tc.tile_pool(name="psv", bufs=1, space="PSUM") as psv,
        ):
            # ---- packed small inputs (SP ring) ----
            pa = fk.tile([P96, PA_COLS], F32)
            nc.sync.dma_start(out=pa, in_=d_pa[:])
            pb = fk.tile([NB + 2, PB_COLS], F32)
            nc.sync.dma_start(out=pb, in_=d_pb[:])
            th = pa[:, 0:3 * K].rearrange("p (k j) -> p k j", j=3)
            jt = pa[:, 3 * K:6 * K].rearrange("p (k j) -> p k j", j=3)
            screp = pa[:, 6 * K:6 * K + 1]
            trfl = pa[:, 6 * K + 1:6 * K + 2]
            idrow = pa[:, 6 * K + 2:6 * K + 6]
            ident = pa[:, 6 * K + 6:6 * K + 6 + P96]
            fp = pb[:, 0:P96]
            mmap = pb[:, P96:PB_COLS]

            # ---- weight matrix halves (SP ring), contiguous pieces ----
            wt0 = big.tile([PADC, V], WT_DT)
            wt1 = big.tile([PADC, V], WT_DT)
            for a, b_ in zip(WT_PIECES[:-1], WT_PIECES[1:]):
                pc = b_ - a
                for wt, dw in ((wt0, d_w0), (wt1, d_w1)):
                    nc.sync.dma_start(
                        out=wt[:, a:b_],
                        in_=dw[a * PADC:b_ * PADC].rearrange(
                            "(p n) -> p n", n=pc))

            # ---- sin / cos interleaved so entry pairs batch into single DVE
            # ops: scc cols cz@0, sz@3, cy@4, sy@5, sx@7, cx@8
            halfpi = fk.tile([P96, 1], F32)
            nc.vector.memset(halfpi, float(np.pi / 2))
            scc = fk.tile([P96, K, 9], F32)
            nc.scalar.activation(scc[:, :, 3:8:2], th, AF.Sin)
            # cos(x) = sin(pi/2 - x); |x| <~ 1 keeps the arg in Sin's range
            nc.scalar.activation(scc[:, :, 0:9:4], th, AF.Sin,
                                 bias=halfpi[:, 0:1], scale=-1.0)

            # ---- bone-length factors -> joint offsets (one-hot matmul) ----
            # fp rows 0..27: tanh(blc/10) (2*sigmoid(x/5) = 1 + tanh(x/10);
            # the +1 is folded into mmap's ones-row); row 28: ones; row 29:
            # center_bone_length. Tanh shares the ACT table set with Sin.
            nc.scalar.activation(fp[0:NB, :], fp[0:NB, :], AF.Tanh, scale=0.1)
            ps_off = ps1.tile([P96, K, 3], F32)
            nc.tensor.matmul(ps_off[:].rearrange("p k j -> p (k j)"), fp, mmap,
                             start=True, stop=True)

            # ---- A matrices: Abuf[p, k, m*4+c]; rows m replicated over p%3 ----
            # batched pair/quad ops over the interleaved sin/cos tile
            ab = fk.tile([P96, KP, 12], F32)
            nc.vector.memset(ab[:, K:KP, :], 0.0)      # phantom joint 62
            cy_b = scc[:, :, 4:5].to_broadcast([P96, K, 2])
            sy_b = scc[:, :, 5:6].to_broadcast([P96, K, 2])
            zpair = scc[:, :, 0:4:3]                    # (cz, sz)
            xpair = scc[:, :, 7:9]                      # (sx, cx)
            t12 = tmp.tile([P96, K, 2], F32)
            nc.vector.tensor_mul(t12, zpair, sy_b)      # (t1, t2) = (cz, sz)*sy
            nc.vector.tensor_mul(ab[:, 0:K, 0:5:4], zpair, cy_b)   # A00, A10
            nc.vector.tensor_scalar_mul(ab[:, 0:K, 8], scc[:, :, 5], -1.0)  # A20
            nc.vector.tensor_mul(ab[:, 0:K, 9:11], xpair, cy_b)    # A21, A22
            # P = (t1sx, t1cx, t2sx, t2cx); Q = (czsx, szsx, czcx, szcx)
            pq = tmp.tile([P96, K, 2, 2], F32, tag="pq")
            qq = tmp.tile([P96, K, 2, 2], F32, tag="qq")
            nc.vector.tensor_mul(
                pq, t12[:].unsqueeze(3).to_broadcast([P96, K, 2, 2]),
                xpair.unsqueeze(2).to_broadcast([P96, K, 2, 2]))
            nc.vector.tensor_mul(
                qq, zpair.unsqueeze(2).to_broadcast([P96, K, 2, 2]),
                xpair.unsqueeze(3).to_broadcast([P96, K, 2, 2]))
            pf = pq[:].rearrange("p k a b -> p k (a b)")
            qf = qq[:].rearrange("p k a b -> p k (a b)")
            # A02 = t1cx + szsx ; A11 = t2sx + czcx  (one paired add)
            nc.vector.tensor_add(ab[:, 0:K, 2:6:3], pf[:, :, 1:3], qf[:, :, 1:3])
            nc.vector.tensor_sub(ab[:, 0:K, 1], pf[:, :, 0], qf[:, :, 3])  # A01
            nc.vector.tensor_sub(ab[:, 0:K, 6], pf[:, :, 3], qf[:, :, 0])  # A12
            # translation column m*4+3 from the one-hot matmul
            nc.vector.tensor_copy(ab[:, 0:K, 3::4], ps_off)

            # ---- FK compose; Gbuf[p, k, c] holds row (p%3) of G[b, k] ----
            g = fk.tile([P96, KP, 4], F32)
            lt4 = tmp.tile([P96, 16, 2, 4], F32, tag="lt4")
            # level 0: virtual identity parent selects row r = p%3 of A[0]
            acc0 = g[:, 0:1, :]
            nc.vector.tensor_mul(
                acc0, ab[:, 0:1, 0:4],
                idrow[:, 0:1].unsqueeze(2).to_broadcast([P96, 1, 4]))
            nc.vector.tensor_mul(
                lt4[:, 0:1, 0, :], ab[:, 0:1, 4:8],
                idrow[:, 1:2].unsqueeze(2).to_broadcast([P96, 1, 4]))
            nc.vector.tensor_add(acc0, acc0, lt4[:, 0:1, 0, :])
            nc.vector.tensor_mul(
                lt4[:, 0:1, 0, :], ab[:, 0:1, 8:12],
                idrow[:, 2:3].unsqueeze(2).to_broadcast([P96, 1, 4]))
            nc.vector.tensor_add(acc0, acc0, lt4[:, 0:1, 0, :])

            jb = fk.tile([P96, K], F32)
            gflat = g[:].rearrange("p k c -> p (k c)")
            lh0 = fk.tile([PADC, 128], WT_DT)
            lh1 = fk.tile([PADC, 128], WT_DT)
            nc.vector.memset(lh0, 0.0)
            nc.vector.memset(lh1, 0.0)

            def compose_level(s, n, ps):
                t = n // 2
                gs4 = g[:, s:s + n, :].rearrange("p (t d) c -> p t d c", d=2)
                as4 = ab[:, s:s + n, :].rearrange("p (t d) e -> p t d e", d=2)
                lts = lt4[:, 0:t, :, :]

                def gp(m):
                    return g[:, ps:ps + t, m:m + 1].unsqueeze(2).to_broadcast(
                        [P96, t, 2, 4])
                nc.vector.tensor_mul(gs4, as4[:, :, :, 0:4], gp(0))
                nc.vector.tensor_mul(lts, as4[:, :, :, 4:8], gp(1))
                nc.vector.tensor_add(gs4, gs4, lts)
                nc.vector.tensor_mul(lts, as4[:, :, :, 8:12], gp(2))
                nc.vector.tensor_add(gs4, gs4, lts)
                nc.vector.tensor_add(
                    g[:, s:s + n, 3].rearrange("p (t d) -> p t d", d=2),
                    g[:, s:s + n, 3].rearrange("p (t d) -> p t d", d=2),
                    g[:, ps:ps + t, 3:4].to_broadcast([P96, t, 2]))

            g2 = fk.tile([P96, K, 4], F32)
            g2flat = g2[:].rearrange("p k c -> p (k c)")

            def finish_half(h, lo, hi, pt_tag, lh):
                """J extract + scale/trans fold into g2 (the inverse bind is
                folded into the host-side weight matrix), transpose + cast
                joints [lo, hi) -> lhsT half."""
                nk = hi - lo
                nc.vector.tensor_scalar(
                    jb[:, lo:hi], g[:, lo:hi, 3], screp, trfl,
                    mybir.AluOpType.mult, mybir.AluOpType.add)
                nc.vector.tensor_scalar_mul(g2[:, lo:hi, :],
                                            g[:, lo:hi, :], screp)
                nc.vector.tensor_scalar(g2[:, lo:hi, 3], g[:, lo:hi, 3],
                                        screp, trfl,
                                        mybir.AluOpType.mult,
                                        mybir.AluOpType.add)
                gfl = g2flat[:, lo * 4:hi * 4]
                pt = ps1.tile([4 * nk, P96], F32, tag="pt")
                nc.tensor.transpose(pt, gfl, ident)
                nc.scalar.copy(lh[0:4 * nk, 0:P96], pt)

            for (s, n, ps) in LEVELS[:4]:
                compose_level(s, n, ps)
            finish_half(0, 0, 31, "pt0", lh0)
            compose_level(*LEVELS[4])
            finish_half(1, 31, 62, "pt1", lh1)
            nc.gpsimd.dma_start(out=d_jo[:], in_=jb)

            # ---- skinning matmul, streamed over V; stores on ACT ring ----
            pvs = [psv.tile([PADC, CHUNK], F32, name=f"pv{i}",
                                  tag=f"pv{i}") for i in range(6)]
            for pi, (c0, c1) in enumerate(OUT_PIECES):
                a, b_ = c0 * CHUNK, min(c1 * CHUNK, V)
                vop = big.tile([P96, b_ - a], F32, tag=f"vop{pi}")
                for ci in range(c0, c1):
                    off = ci * CHUNK
                    n = min(CHUNK, V - off)
                    pv = pvs[ci % 6]
                    nc.tensor.matmul(pv[:, 0:n], lh0, wt0[:, off:off + n],
                                     start=True, stop=False)
                    nc.tensor.matmul(pv[:, 0:n], lh1, wt1[:, off:off + n],
                                     start=False, stop=True)
                    lo = off - a
                    if ci % 2 == 0:
                        nc.scalar.copy(vop[:, lo:lo + n], pv[0:P96, 0:n])
                    else:
                        nc.vector.tensor_copy(vop[:, lo:lo + n], pv[0:P96, 0:n])
                # issue stores from engines with no copy work (SP ring is
                # idle once loads finish; GPSIMD SWDGE has its own queues)
                eng = nc.sync if pi % 2 == 0 else nc.gpsimd
                eng.dma_start(
                    out=d_vo[a * P96:b_ * P96].rearrange(
                        "(p n) -> p n", n=b_ - a),
                    in_=vop)

    _split_sync_waits(nc)
    return nc


_NC = None


def _get_nc():
    global _NC
    if _NC is None:
        _NC = _build()
    return _NC


def _prep(inputs):
    thetas = np.ascontiguousarray(inputs["thetas"], np.float32)
    blc = np.ascontiguousarray(inputs["bone_lengths_core"], np.float32)
    cbl = np.ascontiguousarray(inputs["center_bone_length"], np.float32)
    trans = np.ascontiguousarray(inputs["trans"], np.float32)
    scale = np.ascontiguousarray(inputs["scale"], np.float32)
    vt = np.ascontiguousarray(inputs["v_template"], np.float32)
    Jt = np.ascontiguousarray(inputs["t_pose_joints"], np.float32)
    w = np.ascontiguousarray(inputs["weights"], np.float32)
    parents = np.asarray(inputs["parents"]).astype(np.int64)
    mapper = np.asarray(inputs["bone_length_mapper"]).astype(np.int64)

    assert np.array_equal(parents, np.maximum((np.arange(K) - 1) // 2, 0)), \
        "kernel specialized for the binary kinematic tree"

    # WT2[(k,c), v] = weights[v,k] * (rest_h[v,c] - [Jt[k];0][c])
    # (the inverse bind G[:, :, 3] -= G[:, :, :3] @ Jt[k] is folded in here)
    rest = np.concatenate([vt, np.ones((V, 1), np.float32)], axis=1)  # [V, 4]
    Jth = np.concatenate([Jt, np.zeros((K, 1), np.float32)], axis=1)  # [K, 4]
    WT = (w.T[:, None, :] * (rest.T[None, :, :] - Jth[:, :, None])
          ).reshape(C248, V)

    # pad halves to 128 rows and pack column-pieces contiguously:
    # piece i occupies dram[a*128 : b*128] laid out as [128, b-a] row-major
    def _pack_half(half):
        padded = np.zeros((PADC, V), WT_NP)
        padded[:HALF] = half.astype(WT_NP)
        return np.concatenate(
            [padded[:, a:b_].ravel()
             for a, b_ in zip(WT_PIECES[:-1], WT_PIECES[1:])])
    wt0 = np.ascontiguousarray(_pack_half(WT[:HALF]))
    wt1 = np.ascontiguousarray(_pack_half(WT[HALF:]))

    # one-hot factor/offset matrix (factor = 1 + tanh(blc/10) for mapped
    # bones -> tanh part on the mapper row, +1 part on the ones row)
    offb = Jt - Jt[parents]
    mmap = np.zeros((NB + 2, 3 * K), np.float32)
    for k in range(K):
        for m in range(3):
            col = 3 * k + m
            if k == 0:
                mmap[NB, col] = Jt[0, m]
            elif k == 1:
                mmap[NB + 1, col] = offb[1, m]
            elif mapper[k] < 0:
                mmap[NB, col] = offb[k, m]
            else:
                mmap[mapper[k], col] = offb[k, m]
                mmap[NB, col] = offb[k, m]

    jt_rep = np.tile(Jt.reshape(1, -1), (P96, 1)).astype(np.float32)
    idrow = np.zeros((P96, 4), np.float32)
    idrow[np.arange(P96), np.arange(P96) % 3] = 1.0

    in_maps = []
    for c in range(NCORES):
        s, e = c * BS, (c + 1) * BS
        pa = np.empty((P96, PA_COLS), np.float32)
        pa[:, 0:3 * K] = np.repeat(thetas[s:e], 3, axis=0)
        pa[:, 3 * K:6 * K] = jt_rep
        pa[:, 6 * K] = np.repeat(scale[s:e], 3)
        pa[:, 6 * K + 1] = trans[s:e].reshape(P96)
        pa[:, 6 * K + 2:6 * K + 6] = idrow
        pa[:, 6 * K + 6:6 * K + 6 + P96] = np.eye(P96, dtype=np.float32)
        pb = np.zeros((NB + 2, PB_COLS), np.float32)
        pb[0:NB, 0:P96] = np.repeat(blc[s:e].T, 3, axis=1)
        pb[NB, 0:P96] = 1.0
        pb[NB + 1, 0:P96] = np.repeat(cbl[s:e, 0], 3)
        pb[:, P96:PB_COLS] = mmap
        in_maps.append({
            "pack_a": np.ascontiguousarray(pa),
            "pack_b": np.ascontiguousarray(pb),
            "wt0": wt0, "wt1": wt1,
        })
    return in_maps


def _unpack_v(flat):
    """[96*V] piece-blocked -> [96, V]"""
    outs = []
    for (c0, c1) in OUT_PIECES:
        a, b_ = c0 * CHUNK, min(c1 * CHUNK, V)
        outs.append(flat[a * P96:b_ * P96].reshape(P96, b_ - a))
    return np.concatenate(outs, axis=1)


def _run(inputs, trace=False):
    nc = _get_nc()
    in_maps = _prep(inputs)
    res = run_bass_kernel_spmd(nc, in_maps, core_ids=list(range(NCORES)),
                               trace=trace)
    Vs, Js = [], []
    for r in res.results:
        Vs.append(_unpack_v(r["v_out"]).reshape(BS, 3, V).transpose(0, 2, 1))
        Js.append(r["j_out"].reshape(BS, 3, K).transpose(0, 2, 1))
    V_final = np.ascontiguousarray(np.concatenate(Vs, axis=0), np.float32)
    J_out = np.ascontiguousarray(np.concatenate(Js, axis=0), np.float32)
    return (V_final, J_out), res


def kernel(**inputs):
    out, _ = _run(inputs, trace=False)
    return out


# revision 31
# speedup vs baseline: 1.1249x; 1.1249x over previous
"""Trainium2 Bass kernel for nn_BodyModelTorch (SMPL-style body model).

Strategy (pure data parallel, batch sharded across 8 cores):
  - Per core: B_shard=32 bodies on 96 partitions laid out as p = 3*b + i
    (i = coordinate row), so per-partition scalars (scale, trans) and the
    final output layout fall out naturally.
  - Forward kinematics on device: ACT sin/cos (tanh for bone factors -- one
    shared ACT table set), DVE rotation-matrix entries, one-hot matmul for
    the bone-length factor gather (+ offset scaling), 5-level binary-tree
    compose with partition-replicated A matrices.
  - LBS skinning as a single 256-deep (248 + zero pad) contraction:
        v[b,i,v'] = sum_{k,c} G''[(k,c),(b,i)] * WT'[(k,c),v']
    where WT'[(k,c),v'] = weights[v',k] * rest_h[v',c] is batch-independent
    and precomputed on the host; scale/trans/inverse-bind are folded into
    G'' so there is no epilogue.
  - WT' ships as fp16 (rel err ~1e-4; set WT_FP32=True for full fp32).
  - All big DMAs use contiguous DRAM blocks shaped [128, cols] so the HWDGE
    sprays descriptors over all 16 SDMA engines (strided 2D DRAM transfers
    pin to 4 engines = ~54 GB/s).
  - Loads ride the SP HWDGE ring (nc.sync), stores the ACT ring (nc.scalar).
  - Device output is [96, V] / [96, K] piece-blocked; the host reorders to
    [B, V, 3] / [B, K, 3] while unsharding.
"""

import numpy as np

import concourse.bass as bass
import concourse.tile as tile
from concourse import mybir
from concourse.bass_utils import run_bass_kernel_spmd

F32 = mybir.dt.float32
AF = mybir.ActivationFunctionType

WT_FP32 = False
WT_DT = F32 if WT_FP32 else mybir.dt.float16
WT_NP = np.float32 if WT_FP32 else np.float16
import os as _os
OUT_FP16 = _os.environ.get("OUT_FP16", "0") == "1"
VO_DT = mybir.dt.float16 if OUT_FP16 else F32
VO_NP = np.float16 if OUT_FP16 else np.float32

B, K, V, NB = 256, 62, 14522, 28
NCORES = 8
BS = B // NCORES            # 32 bodies per core
P96 = 3 * BS                # 96 partitions: p = 3*b + i
C248 = 4 * K                # contraction dim (k, c)
HALF = C248 // 2            # 124 = 31 joints * 4
PADC = 128                  # contraction rows per half (zero-padded)
CHUNK = 512
NCHUNK = (V + CHUNK - 1) // CHUNK            # 29
# (start, count, parent_start) per tree level; parents[j] = (j-1)//2.
# level 5 is padded to 32 joints (phantom joint 62 writes scratch).
LEVELS = [(1, 2, 0), (3, 4, 1), (7, 8, 3), (15, 16, 7), (31, 32, 15)]
KP = K + 1                  # joint-padded buffers
WT_PIECES = [0, 2048, 4096, 6144, 8192, 10240, 12288, V]
# output pieces in chunk indices; each piece is its own SBUF tile and its
# own contiguous [96, cols] DRAM block
OUT_PIECES = [(c, min(c + 2, NCHUNK)) for c in range(0, NCHUNK, 2)]


def _split_sync_waits(nc, maxw=1):
    """The walrus build here accepts only one sync-wait command per
    instruction; move surplus waits onto preceding same-engine NOPs."""
    for f in nc.m.functions:
        for bb in f.blocks:
            insts = list(bb.instructions)
            out = []
            changed = False
            for ins in insts:
                si = ins.sync_info
                if si is not None and si.on_wait and len(si.on_wait) > maxw:
                    waits = list(si.on_wait)
                    extra, keep = waits[:-maxw], waits[-maxw:]
                    for j, w in enumerate(extra):
                        out.append(mybir.InstNoOp(
                            name=f"{ins.name}-wsplit{j}",
                            sync_info=mybir.SyncInfo(on_wait=[w], on_update=[]),
                            bass_nofuse=True,
                            engine=ins.engine,
                        ))
                    si.on_wait = keep
                    changed = True
                out.append(ins)
            if changed:
                bb.instructions[:] = out


PA_COLS = 3 * K + 3 * K + 1 + 1 + 4 + P96    # th|jt|scale|trans|idrow|I96
PB_COLS = P96 + 3 * K                         # fpack | mmap


def _build():
    nc = bass.Bass("TRN2")
    d_pa = nc.dram_tensor("pack_a", [P96, PA_COLS], F32, kind="ExternalInput")
    d_pb = nc.dram_tensor("pack_b", [NB + 2, PB_COLS], F32, kind="ExternalInput")
    d_w0 = nc.dram_tensor("wt0", [PADC * V], WT_DT, kind="ExternalInput")
    d_w1 = nc.dram_tensor("wt1", [PADC * V], WT_DT, kind="ExternalInput")
    d_vo = nc.dram_tensor("v_out", [P96 * V], VO_DT, kind="ExternalOutput")
    d_jo = nc.dram_tensor("j_out", [P96, K], F32, kind="ExternalOutput")

    with tile.TileContext(nc) as tc:
        with (
            tc.tile_pool(name="big", bufs=1) as big,
            tc.tile_pool(name="fk", bufs=1) as fk,
            tc.tile_pool(name="tmp", bufs=1) as tmp,
            tc.tile_pool(name="ps1", bufs=1, space="PSUM") as ps1,
            tc.tile_pool(name="psv", bufs=1, space="PSUM") as psv,
        ):
            # ---- packed small inputs (SP ring) ----
            pa = fk.tile([P96, PA_COLS], F32)
            nc.sync.dma_start(out=pa, in_=d_pa[:])
            pb = fk.tile([NB + 2, PB_COLS], F32)
            nc.sync.dma_start(out=pb, in_=d_pb[:])
            th = pa[:, 0:3 * K].rearrange("p (k j) -> p k j", j=3)
            jt = pa[:, 3 * K:6 * K].rearrange("p (k j) -> p k j", j=3)
            screp = pa[:, 6 * K:6 * K + 1]
            trfl = pa[:, 6 * K + 1:6 * K + 2]
            idrow = pa[:, 6 * K + 2:6 * K + 6]
            ident = pa[:, 6 * K + 6:6 * K + 6 + P96]
            fp = pb[:, 0:P96]
            mmap = pb[:, P96:PB_COLS]

            # ---- weight matrix halves (SP ring), contiguous pieces ----
            wt0 = big.tile([PADC, V], WT_DT)
            wt1 = big.tile([PADC, V], WT_DT)
            for a, b_ in zip(WT_PIECES[:-1], WT_PIECES[1:]):
                pc = b_ - a
                for wt, dw in ((wt0, d_w0), (wt1, d_w1)):
                    nc.sync.dma_start(
                        out=wt[:, a:b_],
                        in_=dw[a * PADC:b_ * PADC].rearrange(
                            "(p n) -> p n", n=pc))

            # ---- sin / cos interleaved so entry pairs batch into single DVE
            # ops: scc cols cz@0, sz@3, cy@4, sy@5, sx@7, cx@8
            halfpi = fk.tile([P96, 1], F32)
            nc.vector.memset(halfpi, float(np.pi / 2))
            scc = fk.tile([P96, K, 9], F32)
            nc.scalar.activation(scc[:, :, 3:8:2], th, AF.Sin)
            # cos(x) = sin(pi/2 - x); |x| <~ 1 keeps the arg in Sin's range
            nc.scalar.activation(scc[:, :, 0:9:4], th, AF.Sin,
                                 bias=halfpi[:, 0:1], scale=-1.0)

            # ---- bone-length factors -> joint offsets (one-hot matmul) ----
            # fp rows 0..27: tanh(blc/10) (2*sigmoid(x/5) = 1 + tanh(x/10);
            # the +1 is folded into mmap's ones-row); row 28: ones; row 29:
            # center_bone_length. Tanh shares the ACT table set with Sin.
            nc.scalar.activation(fp[0:NB, :], fp[0:NB, :], AF.Tanh, scale=0.1)
            ps_off = ps1.tile([P96, K, 3], F32)
            nc.tensor.matmul(ps_off[:].rearrange("p k j -> p (k j)"), fp, mmap,
                             start=True, stop=True)

            # ---- A matrices: Abuf[p, k, m*4+c]; rows m replicated over p%3 ----
            # batched pair/quad ops over the interleaved sin/cos tile
            ab = fk.tile([P96, KP, 12], F32)
            nc.vector.memset(ab[:, K:KP, :], 0.0)      # phantom joint 62
            cy_b = scc[:, :, 4:5].to_broadcast([P96, K, 2])
            sy_b = scc[:, :, 5:6].to_broadcast([P96, K, 2])
            zpair = scc[:, :, 0:4:3]                    # (cz, sz)
            xpair = scc[:, :, 7:9]                      # (sx, cx)
            t12 = tmp.tile([P96, K, 2], F32)
            nc.vector.tensor_mul(t12, zpair, sy_b)      # (t1, t2) = (cz, sz)*sy
            nc.vector.tensor_mul(ab[:, 0:K, 0:5:4], zpair, cy_b)   # A00, A10
            nc.vector.tensor_scalar_mul(ab[:, 0:K, 8], scc[:, :, 5], -1.0)  # A20
            nc.vector.tensor_mul(ab[:, 0:K, 9:11], xpair, cy_b)    # A21, A22
            # P = (t1sx, t1cx, t2sx, t2cx); Q = (czsx, szsx, czcx, szcx)
            pq = tmp.tile([P96, K, 2, 2], F32, tag="pq")
            qq = tmp.tile([P96, K, 2, 2], F32, tag="qq")
            nc.vector.tensor_mul(
                pq, t12[:].unsqueeze(3).to_broadcast([P96, K, 2, 2]),
                xpair.unsqueeze(2).to_broadcast([P96, K, 2, 2]))
            nc.vector.tensor_mul(
                qq, zpair.unsqueeze(2).to_broadcast([P96, K, 2, 2]),
                xpair.unsqueeze(3).to_broadcast([P96, K, 2, 2]))
            pf = pq[:].rearrange("p k a b -> p k (a b)")
            qf = qq[:].rearrange("p k a b -> p k (a b)")
            # A02 = t1cx + szsx ; A11 = t2sx + czcx  (one paired add)
            nc.vector.tensor_add(ab[:, 0:K, 2:6:3], pf[:, :, 1:3], qf[:, :, 1:3])
            nc.vector.tensor_sub(ab[:, 0:K, 1], pf[:, :, 0], qf[:, :, 3])  # A01
            nc.vector.tensor_sub(ab[:, 0:K, 6], pf[:, :, 3], qf[:, :, 0])  # A12
            # translation column m*4+3 from the one-hot matmul
            nc.vector.tensor_copy(ab[:, 0:K, 3::4], ps_off)

            # ---- FK compose; Gbuf[p, k, c] holds row (p%3) of G[b, k] ----
            g = fk.tile([P96, KP, 4], F32)
            lt4 = tmp.tile([P96, 16, 2, 4], F32, tag="lt4")
            # level 0: virtual identity parent selects row r = p%3 of A[0]
            acc0 = g[:, 0:1, :]
            nc.vector.tensor_mul(
                acc0, ab[:, 0:1, 0:4],
                idrow[:, 0:1].unsqueeze(2).to_broadcast([P96, 1, 4]))
            nc.vector.tensor_mul(
                lt4[:, 0:1, 0, :], ab[:, 0:1, 4:8],
                idrow[:, 1:2].unsqueeze(2).to_broadcast([P96, 1, 4]))
            nc.vector.tensor_add(acc0, acc0, lt4[:, 0:1, 0, :])
            nc.vector.tensor_mul(
                lt4[:, 0:1, 0, :], ab[:, 0:1, 8:12],
                idrow[:, 2:3].unsqueeze(2).to_broadcast([P96, 1, 4]))
            nc.vector.tensor_add(acc0, acc0, lt4[:, 0:1, 0, :])

            jb = fk.tile([P96, K], F32)
            gflat = g[:].rearrange("p k c -> p (k c)")
            lh0 = fk.tile([PADC, 128], WT_DT)
            lh1 = fk.tile([PADC, 128], WT_DT)
            nc.vector.memset(lh0, 0.0)
            nc.vector.memset(lh1, 0.0)

            def compose_level(s, n, ps):
                t = n // 2
                gs4 = g[:, s:s + n, :].rearrange("p (t d) c -> p t d c", d=2)
                as4 = ab[:, s:s + n, :].rearrange("p (t d) e -> p t d e", d=2)
                lts = lt4[:, 0:t, :, :]

                def gp(m):
                    return g[:, ps:ps + t, m:m + 1].unsqueeze(2).to_broadcast(
                        [P96, t, 2, 4])
                nc.vector.tensor_mul(gs4, as4[:, :, :, 0:4], gp(0))
                nc.vector.tensor_mul(lts, as4[:, :, :, 4:8], gp(1))
                nc.vector.tensor_add(gs4, gs4, lts)
                nc.vector.tensor_mul(lts, as4[:, :, :, 8:12], gp(2))
                nc.vector.tensor_add(gs4, gs4, lts)
                nc.vector.tensor_add(
                    g[:, s:s + n, 3].rearrange("p (t d) -> p t d", d=2),
                    g[:, s:s + n, 3].rearrange("p (t d) -> p t d", d=2),
                    g[:, ps:ps + t, 3:4].to_broadcast([P96, t, 2]))

            g2 = fk.tile([P96, K, 4], F32)
            g2flat = g2[:].rearrange("p k c -> p (k c)")

            def finish_half(h, lo, hi, pt_tag, lh):
                """J extract + scale/trans fold into g2 (the inverse bind is
                folded into the host-side weight matrix), transpose + cast
                joints [lo, hi) -> lhsT half."""
                nk = hi - lo
                nc.vector.tensor_scalar(
                    jb[:, lo:hi], g[:, lo:hi, 3], screp, trfl,
                    mybir.AluOpType.mult, mybir.AluOpType.add)
                nc.vector.tensor_scalar_mul(g2[:, lo:hi, :],
                                            g[:, lo:hi, :], screp)
                nc.vector.tensor_scalar(g2[:, lo:hi, 3], g[:, lo:hi, 3],
                                        screp, trfl,
                                        mybir.AluOpType.mult,
                                        mybir.AluOpType.add)
                gfl = g2flat[:, lo * 4:hi * 4]
                pt = ps1.tile([4 * nk, P96], F32, tag="pt")
                nc.tensor.transpose(pt, gfl, ident)
                nc.scalar.copy(lh[0:4 * nk, 0:P96], pt)

            for (s, n, ps) in LEVELS[:4]:
                compose_level(s, n, ps)
            finish_half(0, 0, 31, "pt0", lh0)
            compose_level(*LEVELS[4])
            finish_half(1, 31, 62, "pt1", lh1)
            nc.gpsimd.dma_start(out=d_jo[:], in_=jb)

            # ---- skinning matmul, streamed over V; stores on ACT ring ----
            pvs = [psv.tile([PADC, CHUNK], F32, name=f"pv{i}",
                                  tag=f"pv{i}") for i in range(6)]
            for pi, (c0, c1) in enumerate(OUT_PIECES):
                a, b_ = c0 * CHUNK, min(c1 * CHUNK, V)
                vop = big.tile([P96, b_ - a], VO_DT, tag=f"vop{pi}")
                for ci in range(c0, c1):
                    off = ci * CHUNK
                    n = min(CHUNK, V - off)
                    pv = pvs[ci % 6]
                    nc.tensor.matmul(pv[:, 0:n], lh0, wt0[:, off:off + n],
                                     start=True, stop=False)
                    nc.tensor.matmul(pv[:, 0:n], lh1, wt1[:, off:off + n],
                                     start=False, stop=True)
                    lo = off - a
                    if ci % 2 == 0:
                        nc.scalar.copy(vop[:, lo:lo + n], pv[0:P96, 0:n])
                    else:
                        nc.vector.tensor_copy(vop[:, lo:lo + n], pv[0:P96, 0:n])
                # issue stores from engines with no copy work (SP ring is
                # idle once loads finish; GPSIMD SWDGE has its own queues)
                eng = nc.sync if pi % 2 == 0 else nc.gpsimd
                eng.dma_start(
                    out=d_vo[a * P96:b_ * P96].rearrange(
                        "(p n) -> p n", n=b_ - a),
                    in_=vop)

    _split_sync_waits(nc)
    return nc


_NC = None


def _get_nc():
    global _NC
    if _NC is None:
        _NC = _build()
    return _NC


def _prep(inputs):
    thetas = np.ascontiguousarray(inputs["thetas"], np.float32)
    blc = np.ascontiguousarray(inputs["bone_lengths_core"], np.float32)
    cbl = np.ascontiguousarray(inputs["center_bone_length"], np.float32)
    trans = np.ascontiguousarray(inputs["trans"], np.float32)
    scale = np.ascontiguousarray(inputs["scale"], np.float32)
    vt = np.ascontiguousarray(inputs["v_template"], np.float32)
    Jt = np.ascontiguousarray(inputs["t_pose_joints"], np.float32)
    w = np.ascontiguousarray(inputs["weights"], np.float32)
    parents = np.asarray(inputs["parents"]).astype(np.int64)
    mapper = np.asarray(inputs["bone_length_mapper"]).astype(np.int64)

    assert np.array_equal(parents, np.maximum((np.arange(K) - 1) // 2, 0)), \
        "kernel specialized for the binary kinematic tree"

    # WT2[(k,c), v] = weights[v,k] * (rest_h[v,c] - [Jt[k];0][c])
    # (the inverse bind G[:, :, 3] -= G[:, :, :3] @ Jt[k] is folded in here)
    rest = np.concatenate([vt, np.ones((V, 1), np.float32)], axis=1)  # [V, 4]
    Jth = np.concatenate([Jt, np.zeros((K, 1), np.float32)], axis=1)  # [K, 4]
    WT = (w.T[:, None, :] * (rest.T[None, :, :] - Jth[:, :, None])
          ).reshape(C248, V)

    # pad halves to 128 rows and pack column-pieces contiguously:
    # piece i occupies dram[a*128 : b*128] laid out as [128, b-a] row-major
    def _pack_half(half):
        padded = np.zeros((PADC, V), WT_NP)
        padded[:HALF] = half.astype(WT_NP)
        return np.concatenate(
            [padded[:, a:b_].ravel()
             for a, b_ in zip(WT_PIECES[:-1], WT_PIECES[1:])])
    wt0 = np.ascontiguousarray(_pack_half(WT[:HALF]))
    wt1 = np.ascontiguousarray(_pack_half(WT[HALF:]))

    # one-hot factor/offset matrix (factor = 1 + tanh(blc/10) for mapped
    # bones -> tanh part on the mapper row, +1 part on the ones row)
    offb = Jt - Jt[parents]
    mmap = np.zeros((NB + 2, 3 * K), np.float32)
    for k in range(K):
        for m in range(3):
            col = 3 * k + m
            if k == 0:
                mmap[NB, col] = Jt[0, m]
            elif k == 1:
                mmap[NB + 1, col] = offb[1, m]
            elif mapper[k] < 0:
                mmap[NB, col] = offb[k, m]
            else:
                mmap[mapper[k], col] = offb[k, m]
                mmap[NB, col] = offb[k, m]

    jt_rep = np.tile(Jt.reshape(1, -1), (P96, 1)).astype(np.float32)
    idrow = np.zeros((P96, 4), np.float32)
    idrow[np.arange(P96), np.arange(P96) % 3] = 1.0

    in_maps = []
    for c in range(NCORES):
        s, e = c * BS, (c + 1) * BS
        pa = np.empty((P96, PA_COLS), np.float32)
        pa[:, 0:3 * K] = np.repeat(thetas[s:e], 3, axis=0)
        pa[:, 3 * K:6 * K] = jt_rep
        pa[:, 6 * K] = np.repeat(scale[s:e], 3)
        pa[:, 6 * K + 1] = trans[s:e].reshape(P96)
        pa[:, 6 * K + 2:6 * K + 6] = idrow
        pa[:, 6 * K + 6:6 * K + 6 + P96] = np.eye(P96, dtype=np.float32)
        pb = np.zeros((NB + 2, PB_COLS), np.float32)
        pb[0:NB, 0:P96] = np.repeat(blc[s:e].T, 3, axis=1)
        pb[NB, 0:P96] = 1.0
        pb[NB + 1, 0:P96] = np.repeat(cbl[s:e, 0], 3)
        pb[:, P96:PB_COLS] = mmap
        in_maps.append({
            "pack_a": np.ascontiguousarray(pa),
            "pack_b": np.ascontiguousarray(pb),
            "wt0": wt0, "wt1": wt1,
        })
    return in_maps


def _unpack_v(flat):
    """[96*V] piece-blocked -> [96, V]"""
    outs = []
    for (c0, c1) in OUT_PIECES:
        a, b_ = c0 * CHUNK, min(c1 * CHUNK, V)
        outs.append(flat[a * P96:b_ * P96].reshape(P96, b_ - a))
    return np.concatenate(outs, axis=1).astype(np.float32)


def _run(inputs, trace=False):
    nc = _get_nc()
    in_maps = _prep(inputs)
    res = run_bass_kernel_spmd(nc, in_maps, core_ids=list(range(NCORES)),
                               trace=trace)
    Vs, Js = [], []
    for r in res.results:
        Vs.append(_unpack_v(r["v_out"]).reshape(BS, 3, V).transpose(0, 2, 1))
        Js.append(r["j_out"].reshape(BS, 3, K).transpose(0, 2, 1))
    V_final = np.ascontiguousarray(np.concatenate(Vs, axis=0), np.float32)
    J_out = np.ascontiguousarray(np.concatenate(Js, axis=0), np.float32)
    return (V_final, J_out), res


def kernel(**inputs):
    out, _ = _run(inputs, trace=False)
    return out


# revision 32
# speedup vs baseline: 1.2399x; 1.1022x over previous
"""Trainium2 Bass kernel for nn_BodyModelTorch (SMPL-style body model).

Strategy (pure data parallel, batch sharded across 8 cores):
  - Per core: B_shard=32 bodies on 96 partitions laid out as p = 3*b + i
    (i = coordinate row), so per-partition scalars (scale, trans) and the
    final output layout fall out naturally.
  - Forward kinematics on device: ACT sin/cos (tanh for bone factors -- one
    shared ACT table set), DVE rotation-matrix entries, one-hot matmul for
    the bone-length factor gather (+ offset scaling), 5-level binary-tree
    compose with partition-replicated A matrices.
  - LBS skinning as a single 256-deep (248 + zero pad) contraction:
        v[b,i,v'] = sum_{k,c} G''[(k,c),(b,i)] * WT'[(k,c),v']
    where WT'[(k,c),v'] = weights[v',k] * rest_h[v',c] is batch-independent
    and precomputed on the host; scale/trans/inverse-bind are folded into
    G'' so there is no epilogue.
  - WT' ships as fp16 (rel err ~1e-4; set WT_FP32=True for full fp32).
  - All big DMAs use contiguous DRAM blocks shaped [128, cols] so the HWDGE
    sprays descriptors over all 16 SDMA engines (strided 2D DRAM transfers
    pin to 4 engines = ~54 GB/s).
  - Loads ride the SP HWDGE ring (nc.sync), stores the ACT ring (nc.scalar).
  - Device output is [96, V] / [96, K] piece-blocked; the host reorders to
    [B, V, 3] / [B, K, 3] while unsharding.
"""

import numpy as np

import concourse.bass as bass
import concourse.tile as tile
from concourse import mybir
from concourse.bass_utils import run_bass_kernel_spmd

F32 = mybir.dt.float32
AF = mybir.ActivationFunctionType

WT_FP32 = False
WT_DT = F32 if WT_FP32 else mybir.dt.float16
WT_NP = np.float32 if WT_FP32 else np.float16
import os as _os
OUT_FP16 = _os.environ.get("OUT_FP16", "0") == "1"
VO_DT = mybir.dt.float16 if OUT_FP16 else F32
VO_NP = np.float16 if OUT_FP16 else np.float32

B, K, V, NB = 256, 62, 14522, 28
NCORES = 8
BS = B // NCORES            # 32 bodies per core
P96 = 3 * BS                # 96 partitions: p = 3*b + i
C248 = 4 * K                # contraction dim (k, c)
HALF = C248 // 2            # 124 = 31 joints * 4
PADC = 128                  # contraction rows per half (zero-padded)
CHUNK = 512
NCHUNK = (V + CHUNK - 1) // CHUNK            # 29
# (start, count, parent_start) per tree level; parents[j] = (j-1)//2.
# level 5 is padded to 32 joints (phantom joint 62 writes scratch).
LEVELS = [(1, 2, 0), (3, 4, 1), (7, 8, 3), (15, 16, 7), (31, 32, 15)]
KP = K + 1                  # joint-padded buffers
WT_PIECES = [0, 2048, 4096, 6144, 8192, 10240, 12288, V]
# output pieces in chunk indices; each piece is its own SBUF tile and its
# own contiguous [96, cols] DRAM block
OUT_PIECES = [(c, min(c + 2, NCHUNK)) for c in range(0, NCHUNK, 2)]


def _split_sync_waits(nc, maxw=1):
    """The walrus build here accepts only one sync-wait command per
    instruction; move surplus waits onto preceding same-engine NOPs."""
    for f in nc.m.functions:
        for bb in f.blocks:
            insts = list(bb.instructions)
            out = []
            changed = False
            for ins in insts:
                si = ins.sync_info
                if si is not None and si.on_wait and len(si.on_wait) > maxw:
                    waits = list(si.on_wait)
                    extra, keep = waits[:-maxw], waits[-maxw:]
                    for j, w in enumerate(extra):
                        out.append(mybir.InstNoOp(
                            name=f"{ins.name}-wsplit{j}",
                            sync_info=mybir.SyncInfo(on_wait=[w], on_update=[]),
                            bass_nofuse=True,
                            engine=ins.engine,
                        ))
                    si.on_wait = keep
                    changed = True
                out.append(ins)
            if changed:
                bb.instructions[:] = out


PA_COLS = 3 * K + 3 * K + 1 + 1 + 4 + P96    # th|jt|scale|trans|idrow|I96
PB_COLS = P96 + 3 * K                         # fpack | mmap


def _build():
    nc = bass.Bass("TRN2")
    d_pa = nc.dram_tensor("pack_a", [P96, PA_COLS], F32, kind="ExternalInput")
    d_pb = nc.dram_tensor("pack_b", [NB + 2, PB_COLS], F32, kind="ExternalInput")
    d_w0 = nc.dram_tensor("wt0", [PADC * V], WT_DT, kind="ExternalInput")
    d_w1 = nc.dram_tensor("wt1", [PADC * V], WT_DT, kind="ExternalInput")
    d_vo = nc.dram_tensor("v_out", [P96 * V], VO_DT, kind="ExternalOutput")
    d_jo = nc.dram_tensor("j_out", [P96, K], F32, kind="ExternalOutput")

    with tile.TileContext(nc) as tc:
        with (
            tc.tile_pool(name="big", bufs=1) as big,
            tc.tile_pool(name="fk", bufs=1) as fk,
            tc.tile_pool(name="tmp", bufs=1) as tmp,
            tc.tile_pool(name="ps1", bufs=1, space="PSUM") as ps1,
            tc.tile_pool(name="psv", bufs=1, space="PSUM") as psv,
        ):
            # ---- packed small inputs (SP ring) ----
            pa = fk.tile([P96, PA_COLS], F32)
            nc.sync.dma_start(out=pa, in_=d_pa[:])
            pb = fk.tile([NB + 2, PB_COLS], F32)
            nc.sync.dma_start(out=pb, in_=d_pb[:])
            th = pa[:, 0:3 * K].rearrange("p (k j) -> p k j", j=3)
            jt = pa[:, 3 * K:6 * K].rearrange("p (k j) -> p k j", j=3)
            screp = pa[:, 6 * K:6 * K + 1]
            trfl = pa[:, 6 * K + 1:6 * K + 2]
            idrow = pa[:, 6 * K + 2:6 * K + 6]
            ident = pa[:, 6 * K + 6:6 * K + 6 + P96]
            fp = pb[:, 0:P96]
            mmap = pb[:, P96:PB_COLS]

            # ---- weight matrix halves (SP ring), contiguous pieces ----
            wt0 = big.tile([PADC, V], WT_DT)
            wt1 = big.tile([PADC, V], WT_DT)
            for a, b_ in zip(WT_PIECES[:-1], WT_PIECES[1:]):
                pc = b_ - a
                for wt, dw in ((wt0, d_w0), (wt1, d_w1)):
                    nc.sync.dma_start(
                        out=wt[:, a:b_],
                        in_=dw[a * PADC:b_ * PADC].rearrange(
                            "(p n) -> p n", n=pc))

            # ---- sin / cos interleaved so entry pairs batch into single DVE
            # ops: scc cols cz@0, sz@3, cy@4, sy@5, sx@7, cx@8
            halfpi = fk.tile([P96, 1], F32)
            nc.vector.memset(halfpi, float(np.pi / 2))
            # dummy activation: pulls the Sin/Tanh ACT table load ahead of
            # the input DMA so the real sin/cos start as soon as data lands
            warmup = tmp.tile([P96, 1], F32)
            nc.scalar.activation(warmup, halfpi, AF.Sin)
            scc = fk.tile([P96, K, 9], F32)
            nc.scalar.activation(scc[:, :, 3:8:2], th, AF.Sin)
            # cos(x) = sin(pi/2 - x); |x| <~ 1 keeps the arg in Sin's range
            nc.scalar.activation(scc[:, :, 0:9:4], th, AF.Sin,
                                 bias=halfpi[:, 0:1], scale=-1.0)

            # ---- bone-length factors -> joint offsets (one-hot matmul) ----
            # fp rows 0..27: tanh(blc/10) (2*sigmoid(x/5) = 1 + tanh(x/10);
            # the +1 is folded into mmap's ones-row); row 28: ones; row 29:
            # center_bone_length. Tanh shares the ACT table set with Sin.
            nc.scalar.activation(fp[0:NB, :], fp[0:NB, :], AF.Tanh, scale=0.1)
            ps_off = ps1.tile([P96, K, 3], F32)
            nc.tensor.matmul(ps_off[:].rearrange("p k j -> p (k j)"), fp, mmap,
                             start=True, stop=True)

            # ---- A matrices: Abuf[p, k, m*4+c]; rows m replicated over p%3 ----
            # batched pair/quad ops over the interleaved sin/cos tile
            ab = fk.tile([P96, KP, 12], F32)
            nc.vector.memset(ab[:, K:KP, :], 0.0)      # phantom joint 62
            cy_b = scc[:, :, 4:5].to_broadcast([P96, K, 2])
            sy_b = scc[:, :, 5:6].to_broadcast([P96, K, 2])
            zpair = scc[:, :, 0:4:3]                    # (cz, sz)
            xpair = scc[:, :, 7:9]                      # (sx, cx)
            t12 = tmp.tile([P96, K, 2], F32)
            nc.vector.tensor_mul(t12, zpair, sy_b)      # (t1, t2) = (cz, sz)*sy
            nc.vector.tensor_mul(ab[:, 0:K, 0:5:4], zpair, cy_b)   # A00, A10
            nc.vector.tensor_scalar_mul(ab[:, 0:K, 8], scc[:, :, 5], -1.0)  # A20
            nc.vector.tensor_mul(ab[:, 0:K, 9:11], xpair, cy_b)    # A21, A22
            # P = (t1sx, t1cx, t2sx, t2cx); Q = (czsx, szsx, czcx, szcx)
            pq = tmp.tile([P96, K, 2, 2], F32, tag="pq")
            qq = tmp.tile([P96, K, 2, 2], F32, tag="qq")
            nc.vector.tensor_mul(
                pq, t12[:].unsqueeze(3).to_broadcast([P96, K, 2, 2]),
                xpair.unsqueeze(2).to_broadcast([P96, K, 2, 2]))
            nc.vector.tensor_mul(
                qq, zpair.unsqueeze(2).to_broadcast([P96, K, 2, 2]),
                xpair.unsqueeze(3).to_broadcast([P96, K, 2, 2]))
            pf = pq[:].rearrange("p k a b -> p k (a b)")
            qf = qq[:].rearrange("p k a b -> p k (a b)")
            # A02 = t1cx + szsx ; A11 = t2sx + czcx  (one paired add)
            nc.vector.tensor_add(ab[:, 0:K, 2:6:3], pf[:, :, 1:3], qf[:, :, 1:3])
            nc.vector.tensor_sub(ab[:, 0:K, 1], pf[:, :, 0], qf[:, :, 3])  # A01
            nc.vector.tensor_sub(ab[:, 0:K, 6], pf[:, :, 3], qf[:, :, 0])  # A12
            # translation column m*4+3 from the one-hot matmul
            nc.vector.tensor_copy(ab[:, 0:K, 3::4], ps_off)

            # ---- FK compose; Gbuf[p, k, c] holds row (p%3) of G[b, k] ----
            g = fk.tile([P96, KP, 4], F32)
            lt4 = tmp.tile([P96, 16, 2, 4], F32, tag="lt4")
            # level 0: virtual identity parent selects row r = p%3 of A[0]
            acc0 = g[:, 0:1, :]
            nc.vector.tensor_mul(
                acc0, ab[:, 0:1, 0:4],
                idrow[:, 0:1].unsqueeze(2).to_broadcast([P96, 1, 4]))
            nc.vector.tensor_mul(
                lt4[:, 0:1, 0, :], ab[:, 0:1, 4:8],
                idrow[:, 1:2].unsqueeze(2).to_broadcast([P96, 1, 4]))
            nc.vector.tensor_add(acc0, acc0, lt4[:, 0:1, 0, :])
            nc.vector.tensor_mul(
                lt4[:, 0:1, 0, :], ab[:, 0:1, 8:12],
                idrow[:, 2:3].unsqueeze(2).to_broadcast([P96, 1, 4]))
            nc.vector.tensor_add(acc0, acc0, lt4[:, 0:1, 0, :])

            jb = fk.tile([P96, K], F32)
            gflat = g[:].rearrange("p k c -> p (k c)")
            lh0 = fk.tile([PADC, 128], WT_DT)
            lh1 = fk.tile([PADC, 128], WT_DT)
            nc.vector.memset(lh0, 0.0)
            nc.vector.memset(lh1, 0.0)

            def compose_level(s, n, ps):
                t = n // 2
                gs4 = g[:, s:s + n, :].rearrange("p (t d) c -> p t d c", d=2)
                as4 = ab[:, s:s + n, :].rearrange("p (t d) e -> p t d e", d=2)
                lts = lt4[:, 0:t, :, :]

                def gp(m):
                    return g[:, ps:ps + t, m:m + 1].unsqueeze(2).to_broadcast(
                        [P96, t, 2, 4])
                nc.vector.tensor_mul(gs4, as4[:, :, :, 0:4], gp(0))
                nc.vector.tensor_mul(lts, as4[:, :, :, 4:8], gp(1))
                nc.vector.tensor_add(gs4, gs4, lts)
                nc.vector.tensor_mul(lts, as4[:, :, :, 8:12], gp(2))
                nc.vector.tensor_add(gs4, gs4, lts)
                nc.vector.tensor_add(
                    g[:, s:s + n, 3].rearrange("p (t d) -> p t d", d=2),
                    g[:, s:s + n, 3].rearrange("p (t d) -> p t d", d=2),
                    g[:, ps:ps + t, 3:4].to_broadcast([P96, t, 2]))

            g2 = fk.tile([P96, K, 4], F32)
            g2flat = g2[:].rearrange("p k c -> p (k c)")

            def finish_half(h, lo, hi, pt_tag, lh):
                """J extract + scale/trans fold into g2 (the inverse bind is
                folded into the host-side weight matrix), transpose + cast
                joints [lo, hi) -> lhsT half."""
                nk = hi - lo
                nc.vector.tensor_scalar_mul(g2[:, lo:hi, :],
                                            g[:, lo:hi, :], screp)
                nc.vector.tensor_scalar(g2[:, lo:hi, 3], g[:, lo:hi, 3],
                                        screp, trfl,
                                        mybir.AluOpType.mult,
                                        mybir.AluOpType.add)
                gfl = g2flat[:, lo * 4:hi * 4]
                pt = ps1.tile([4 * nk, P96], F32, tag="pt")
                nc.tensor.transpose(pt, gfl, ident)
                nc.scalar.copy(lh[0:4 * nk, 0:P96], pt)

            for (s, n, ps) in LEVELS[:4]:
                compose_level(s, n, ps)
            finish_half(0, 0, 31, "pt0", lh0)
            compose_level(*LEVELS[4])
            finish_half(1, 31, 62, "pt1", lh1)
            # joints output off the critical path (doesn't gate the stream)
            nc.vector.tensor_scalar(jb, g[:, 0:K, 3], screp, trfl,
                                    mybir.AluOpType.mult, mybir.AluOpType.add)
            nc.gpsimd.dma_start(out=d_jo[:], in_=jb)

            # ---- skinning matmul, streamed over V; stores on ACT ring ----
            pvs = [psv.tile([PADC, CHUNK], F32, name=f"pv{i}",
                                  tag=f"pv{i}") for i in range(6)]
            for pi, (c0, c1) in enumerate(OUT_PIECES):
                a, b_ = c0 * CHUNK, min(c1 * CHUNK, V)
                vop = big.tile([P96, b_ - a], VO_DT, tag=f"vop{pi}")
                for ci in range(c0, c1):
                    off = ci * CHUNK
                    n = min(CHUNK, V - off)
                    pv = pvs[ci % 6]
                    nc.tensor.matmul(pv[:, 0:n], lh0, wt0[:, off:off + n],
                                     start=True, stop=False)
                    nc.tensor.matmul(pv[:, 0:n], lh1, wt1[:, off:off + n],
                                     start=False, stop=True)
                    lo = off - a
                    if ci % 2 == 0:
                        nc.scalar.copy(vop[:, lo:lo + n], pv[0:P96, 0:n])
                    else:
                        nc.vector.tensor_copy(vop[:, lo:lo + n], pv[0:P96, 0:n])
                # issue stores from engines with no copy work (SP ring is
                # idle once loads finish; GPSIMD SWDGE has its own queues)
                eng = nc.sync if pi % 2 == 0 else nc.gpsimd
                eng.dma_start(
                    out=d_vo[a * P96:b_ * P96].rearrange(
                        "(p n) -> p n", n=b_ - a),
                    in_=vop)

    _split_sync_waits(nc)
    return nc


_NC = None


def _get_nc():
    global _NC
    if _NC is None:
        _NC = _build()
    return _NC


def _prep(inputs):
    thetas = np.ascontiguousarray(inputs["thetas"], np.float32)
    blc = np.ascontiguousarray(inputs["bone_lengths_core"], np.float32)
    cbl = np.ascontiguousarray(inputs["center_bone_length"], np.float32)
    trans = np.ascontiguousarray(inputs["trans"], np.float32)
    scale = np.ascontiguousarray(inputs["scale"], np.float32)
    vt = np.ascontiguousarray(inputs["v_template"], np.float32)
    Jt = np.ascontiguousarray(inputs["t_pose_joints"], np.float32)
    w = np.ascontiguousarray(inputs["weights"], np.float32)
    parents = np.asarray(inputs["parents"]).astype(np.int64)
    mapper = np.asarray(inputs["bone_length_mapper"]).astype(np.int64)

    assert np.array_equal(parents, np.maximum((np.arange(K) - 1) // 2, 0)), \
        "kernel specialized for the binary kinematic tree"

    # WT2[(k,c), v] = weights[v,k] * (rest_h[v,c] - [Jt[k];0][c])
    # (the inverse bind G[:, :, 3] -= G[:, :, :3] @ Jt[k] is folded in here)
    rest = np.concatenate([vt, np.ones((V, 1), np.float32)], axis=1)  # [V, 4]
    Jth = np.concatenate([Jt, np.zeros((K, 1), np.float32)], axis=1)  # [K, 4]
    WT = (w.T[:, None, :] * (rest.T[None, :, :] - Jth[:, :, None])
          ).reshape(C248, V)

    # pad halves to 128 rows and pack column-pieces contiguously:
    # piece i occupies dram[a*128 : b*128] laid out as [128, b-a] row-major
    def _pack_half(half):
        padded = np.zeros((PADC, V), WT_NP)
        padded[:HALF] = half.astype(WT_NP)
        return np.concatenate(
            [padded[:, a:b_].ravel()
             for a, b_ in zip(WT_PIECES[:-1], WT_PIECES[1:])])
    wt0 = np.ascontiguousarray(_pack_half(WT[:HALF]))
    wt1 = np.ascontiguousarray(_pack_half(WT[HALF:]))

    # one-hot factor/offset matrix (factor = 1 + tanh(blc/10) for mapped
    # bones -> tanh part on the mapper row, +1 part on the ones row)
    offb = Jt - Jt[parents]
    mmap = np.zeros((NB + 2, 3 * K), np.float32)
    for k in range(K):
        for m in range(3):
            col = 3 * k + m
            if k == 0:
                mmap[NB, col] = Jt[0, m]
            elif k == 1:
                mmap[NB + 1, col] = offb[1, m]
            elif mapper[k] < 0:
                mmap[NB, col] = offb[k, m]
            else:
                mmap[mapper[k], col] = offb[k, m]
                mmap[NB, col] = offb[k, m]

    jt_rep = np.tile(Jt.reshape(1, -1), (P96, 1)).astype(np.float32)
    idrow = np.zeros((P96, 4), np.float32)
    idrow[np.arange(P96), np.arange(P96) % 3] = 1.0

    in_maps = []
    for c in range(NCORES):
        s, e = c * BS, (c + 1) * BS
        pa = np.empty((P96, PA_COLS), np.float32)
        pa[:, 0:3 * K] = np.repeat(thetas[s:e], 3, axis=0)
        pa[:, 3 * K:6 * K] = jt_rep
        pa[:, 6 * K] = np.repeat(scale[s:e], 3)
        pa[:, 6 * K + 1] = trans[s:e].reshape(P96)
        pa[:, 6 * K + 2:6 * K + 6] = idrow
        pa[:, 6 * K + 6:6 * K + 6 + P96] = np.eye(P96, dtype=np.float32)
        pb = np.zeros((NB + 2, PB_COLS), np.float32)
        pb[0:NB, 0:P96] = np.repeat(blc[s:e].T, 3, axis=1)
        pb[NB, 0:P96] = 1.0
        pb[NB + 1, 0:P96] = np.repeat(cbl[s:e, 0], 3)
        pb[:, P96:PB_COLS] = mmap
        in_maps.append({
            "pack_a": np.ascontiguousarray(pa),
            "pack_b": np.ascontiguousarray(pb),
            "wt0": wt0, "wt1": wt1,
        })
    return in_maps


def _unpack_v(flat):
    """[96*V] piece-blocked -> [96, V]"""
    outs = []
    for (c0, c1) in OUT_PIECES:
        a, b_ = c0 * CHUNK, min(c1 * CHUNK, V)
        outs.append(flat[a * P96:b_ * P96].reshape(P96, b_ - a))
    return np.concatenate(outs, axis=1).astype(np.float32)


def _run(inputs, trace=False):
    nc = _get_nc()
    in_maps = _prep(inputs)
    res = run_bass_kernel_spmd(nc, in_maps, core_ids=list(range(NCORES)),
                               trace=trace)
    Vs, Js = [], []
    for r in res.results:
        Vs.append(_unpack_v(r["v_out"]).reshape(BS, 3, V).transpose(0, 2, 1))
        Js.append(r["j_out"].reshape(BS, 3, K).transpose(0, 2, 1))
    V_final = np.ascontiguousarray(np.concatenate(Vs, axis=0), np.float32)
    J_out = np.ascontiguousarray(np.concatenate(Js, axis=0), np.float32)
    return (V_final, J_out), res


def kernel(**inputs):
    out, _ = _run(inputs, trace=False)
    return out
